# revision 51
# baseline (speedup 1.0000x reference)
"""Multi-head attention Trainium2 kernel v3 (8 NeuronCores, SPMD).

Sharding: core c handles batch b=c//2, query-row half r=c%2 (SQ=1024 q rows),
all 8 heads. K/V for the batch are recomputed on both cores of a pair.

v3 redesign vs v2 (570 us baseline -> ~431-445 us): the PE array is the
bottleneck and the whole core's clock throttles under sustained load
(observed 215-550 ns for the same N=512 matmul), so v3 keeps the PE stream
gap-free and minimizes matmul columns:
  - masked rowmax IS required (the measured global-max vs masked-max gap
    reaches 218 on these inputs, beyond any exp-bias rescue window ~137),
    so the id @ mbias add stays, with an fp8e5 bias mask (-57344 exact;
    half the DMA bytes of bf16).
  - dummy bf16 matmuls at t=0 keep the PE busy while the first DMAs land;
    biases are DMA'd FIRST (they gate the proj ACT drains).
  - fine software pipeline: PV(c-1) issues after S^T(c) (never blocks on
    exp/AND latency); stats(h+1) j-steps fill chunks 0-7, its transpose
    tail chunk 8; head h-1 out-proj fills chunks 9-15; the head tail is
    decoupled by ACT drains of the PV PSUM that also remap partitions so
    out-proj runs K=128 (4 matmuls/head instead of 8 K=64 ones).
  - PSUM: sp(2 banks, outer) | proj: pps(4)+dummy(1) | attn: st(2)+ot(2)+
    tr/po(2) = 8 banks with no overcommit.
  - DMA priority order: biases -> wq,xq -> wk,xk -> mb(j<4) -> wv,xv ->
    mb rest -> wo; mt deferred to the attention phase (SBUF headroom).
Output rows are (head, t) blocks; host reassembles the full [B,S,E] tensor.

Notes from failed experiments (do not retry blindly):
  - nc.vector.tensor_tensor_reduce crashes the exec unit on this HW
    (NRT_EXEC_UNIT_UNRECOVERABLE) in every variant, incl. all-f32 SBUF.
  - fp8 DoubleRow on HW does NOT match the interpreter semantics (second
    weight subtile is not applied); plain fp8 operands stream no faster
    than fp16 in practice (per-op time is clock-bound, not dtype-bound).
  - cross-partition-base writes work on ACT but NOT on DVE (silent
    wrong results on HW, correct in CoreSim).
"""

import os
import numpy as np
import ml_dtypes

import concourse.bass as bass
import concourse.mybir as mybir
from concourse import bacc
from concourse.bass_utils import run_bass_kernel_spmd
from concourse.tile import TileContext
from concourse.masks import make_identity

F32 = mybir.dt.float32
F16 = mybir.dt.float16
F32R = mybir.dt.float32r
I16 = mybir.dt.int16
BF16 = mybir.dt.bfloat16
F8E5 = mybir.dt.float8e5
AF = mybir.ActivationFunctionType
ALU = mybir.AluOpType

B, S, E, H, DK = 4, 2048, 512, 8, 64
SQ = S // 2          # q rows per core
NEG = -1000000000.0
N_CORES = 8

NQT = SQ // 128      # 8 q(j) tiles
NKT = S // 128       # 16 k tiles
NE = E // 128        # 4 embed chunks

BF = ml_dtypes.bfloat16


def build_nc():
    _NDUM = int(os.environ.get("KDUM", "40"))
    nc = bacc.Bacc(None, target_bir_lowering=False)

    xqT = nc.declare_dram_parameter("xqT", [E, SQ], F32R, isOutput=False)
    xkT = nc.declare_dram_parameter("xkT", [E, S], F32R, isOutput=False)
    xvT = nc.declare_dram_parameter("xvT", [E, S], BF16, isOutput=False)
    mt = nc.declare_dram_parameter("mt", [S, SQ], I16, isOutput=False)
    mb = nc.declare_dram_parameter("mb", [SQ, S], F8E5, isOutput=False)
    wqT = nc.declare_dram_parameter("wqT", [E, E], F32R, isOutput=False)
    wkT = nc.declare_dram_parameter("wkT", [E, E], F32R, isOutput=False)
    wvT = nc.declare_dram_parameter("wvT", [E, E], BF16, isOutput=False)
    woT = nc.declare_dram_parameter("woT", [E, E], BF16, isOutput=False)
    bqt = nc.declare_dram_parameter("bqt", [128, NE], F32, isOutput=False)
    bkt = nc.declare_dram_parameter("bkt", [128, NE], F32, isOutput=False)
    bvr = nc.declare_dram_parameter("bvr", [128, E], F32, isOutput=False)
    bor = nc.declare_dram_parameter("bor", [128, E], F32, isOutput=False)
    out = nc.declare_dram_parameter("out", [SQ, E], F32, isOutput=True)

    with TileContext(nc) as tc:
        with (
            tc.tile_pool(name="const", bufs=1) as constp,
            tc.tile_pool(name="wo_p", bufs=1) as wo_p,
            tc.tile_pool(name="qk", bufs=1) as qkp,
            tc.tile_pool(name="vp", bufs=1) as vp,
            tc.tile_pool(name="mbp", bufs=1) as mbp,
            tc.tile_pool(name="stat", bufs=2) as statp,
            tc.tile_pool(name="sp_ps", bufs=2, space="PSUM") as sp_ps,
        ):
            # -- PE ramp dummies: first PE instructions, no DMA deps --
            dz = constp.tile([128, 512], BF16, tag="dz", name="dz")
            nc.vector.memset(dz[:, :], 0.0)

            id16 = constp.tile([128, 128], F16, tag="id16", name="id16")
            make_identity(nc, id16[:, :])
            id8 = constp.tile([128, 128], F8E5, tag="id8", name="id8")
            make_identity(nc, id8[:, :])

            wo16 = [wo_p.tile([128, E], BF16, tag=f"wo16_{m}",
                              name=f"wo16_{m}")
                    for m in range(4)]
            bq_sb = constp.tile([128, NE], F32, tag="bq", name="bq")
            bk_sb = constp.tile([128, NE], F32, tag="bk", name="bk")
            bv_sb = constp.tile([128, E], F32, tag="bv", name="bv")
            bo_sb = constp.tile([128, E], F32, tag="bo", name="bo")

            # per-head augmented tiles
            qst = [qkp.tile([65, SQ], F16, tag=f"qst{h}", name=f"qst{h}")
                   for h in range(H)]
            kst = [qkp.tile([65, S], F16, tag=f"kst{h}", name=f"kst{h}")
                   for h in range(H)]
            for h in range(H):
                nc.vector.memset(kst[h][64:65, :], 1.0)
            # V chunks with a ones column per head block: [128, 8*65]
            v_sb = [vp.tile([128, 65 * H], BF16, tag=f"v{c}", name=f"v{c}")
                    for c in range(NKT)]
            for c in range(NKT):
                nc.vector.memset(
                    v_sb[c].rearrange("p (h d) -> p h d", d=65)[:, :, 64:65],
                    1.0)

            # additive bias mask [q, k] tiles (for stats masked rowmax);
            # fp8e5: -57344 is exact, halves DMA bytes vs bf16
            mb_sb = [mbp.tile([128, S], F8E5, tag=f"mb{j}", name=f"mb{j}")
                     for j in range(NQT)]

            def stats_ops(h):
                """Masked rowmax for head h: S + bias-mask accumulated in
                PSUM, plain DVE reduce. Yields after each j tile; final
                yield = transpose + row-64 DMAs into qst[h]."""
                nmx = statp.tile([128, NQT], F16, tag=f"nmx{h % 2}",
                                 name=f"nmx{h}")
                for j in range(NQT):
                    mxs = statp.tile([128, 4], F32, tag="mxs",
                                     name=f"mxs{h}_{j}")
                    for kb in range(4):
                        sp = sp_ps.tile([128, 512], F32, tag="sp", name="sp")
                        nc.tensor.matmul(
                            sp[:, :],
                            qst[h][0:64, 128 * j:128 * j + 128],
                            kst[h][0:64, 512 * kb:512 * kb + 512],
                            start=True, stop=False)
                        nc.tensor.matmul(
                            sp[:, :], id8[:, :],
                            mb_sb[j][:, 512 * kb:512 * kb + 512],
                            start=False, stop=True)
                        nc.vector.tensor_reduce(
                            out=mxs[:, kb:kb + 1], in_=sp[:, :],
                            axis=mybir.AxisListType.X, op=ALU.max)
                    nc.vector.tensor_reduce(
                        out=nmx[:, j:j + 1], in_=mxs[:, :],
                        axis=mybir.AxisListType.X, op=ALU.max,
                        negate=True)
                    yield
                tr = tailps.tile([NQT, 128], F16, tag="tr", name=f"tr{h}")
                nc.tensor.transpose(tr[:, :], nmx[:, :], id16[:, :])
                trsb = statp.tile([NQT, 128], F16, tag="trsb",
                                  name=f"trsb{h}")
                nc.vector.tensor_copy(trsb[:, :], tr[:, :])
                for j in range(NQT):
                    nc.sync.dma_start(
                        out=qst[h][64:65, 128 * j:128 * j + 128],
                        in_=trsb[j:j + 1, :])
                yield

            # ---- projection phase (x inputs + qkv weights scoped) ----
            with (
                tc.tile_pool(name="xin", bufs=1) as xin,
                tc.tile_pool(name="wts", bufs=1) as wts,
                tc.tile_pool(name="pps", bufs=2, space="PSUM") as pps,
                tc.tile_pool(name="dum", bufs=1, space="PSUM") as dum_ps,
            ):
                # dummies keep the PE busy+ramping while DMAs land
                dum = dum_ps.tile([128, 512], F32, tag="dum", name="dum")
                for i in range(_NDUM):
                    nc.tensor.matmul(dum[:, :], dz[:, 0:128], dz[:, :],
                                     start=True, stop=True,
                                     skip_group_check=True)

                wq_sb = [wts.tile([128, E], F32R, tag=f"wq{c}", name=f"wq{c}")
                         for c in range(NE)]
                wk_sb = [wts.tile([128, E], F32R, tag=f"wk{c}", name=f"wk{c}")
                         for c in range(NE)]
                wv_sb = [wts.tile([128, E], BF16, tag=f"wv{c}", name=f"wv{c}")
                         for c in range(NE)]
                xq_sb = [xin.tile([128, SQ], F32R, tag=f"xq{c}", name=f"xq{c}")
                         for c in range(NE)]
                xk_sb = [xin.tile([128, S], F32R, tag=f"xk{c}", name=f"xk{c}")
                         for c in range(NE)]
                xv_sb = [xin.tile([128, S], BF16, tag=f"xv{c}", name=f"xv{c}")
                         for c in range(NE)]
                # DMA priority order: biases (tiny, gate the proj ACT
                # drains) first, then Q-proj inputs split in halves across
                # queues, then K, stats masks for j=0,1, then V, the rest.
                nc.sync.dma_start(out=bq_sb[:, :], in_=bqt[:, :])
                nc.sync.dma_start(out=bk_sb[:, :], in_=bkt[:, :])
                nc.sync.dma_start(out=bv_sb[:, :], in_=bvr[:, :])
                nc.sync.dma_start(out=bo_sb[:, :], in_=bor[:, :])
                for c in range(NE):
                    for hf in range(4):
                        sl = slice(256 * hf, 256 * hf + 256)
                        nc.sync.dma_start(
                            out=xq_sb[c][:, sl],
                            in_=xqT[128 * c:128 * c + 128, sl])
                    for hf in range(2):
                        sl = slice(256 * hf, 256 * hf + 256)
                        nc.sync.dma_start(
                            out=wq_sb[c][:, sl],
                            in_=wqT[128 * c:128 * c + 128, sl])
                for c in range(NE):
                    for hf in range(4):
                        sl = slice(512 * hf, 512 * hf + 512)
                        nc.sync.dma_start(
                            out=xk_sb[c][:, sl],
                            in_=xkT[128 * c:128 * c + 128, sl])
                    nc.sync.dma_start(out=wk_sb[c][:, :],
                                      in_=wkT[128 * c:128 * c + 128, :])
                for j in range(4):
                    nc.sync.dma_start(out=mb_sb[j][:, :],
                                      in_=mb[128 * j:128 * j + 128, :])
                for c in range(NE):
                    nc.sync.dma_start(out=xv_sb[c][:, :],
                                      in_=xvT[128 * c:128 * c + 128, :])
                    nc.sync.dma_start(out=wv_sb[c][:, :],
                                      in_=wvT[128 * c:128 * c + 128, :])
                for j in range(4, NQT):
                    nc.sync.dma_start(out=mb_sb[j][:, :],
                                      in_=mb[128 * j:128 * j + 128, :])
                for m in range(4):
                    nc.sync.dma_start(out=wo16[m][:, :],
                                      in_=woT[128 * m:128 * m + 128, :])

                # Q: one [128, SQ] psum per E-chunk m -> heads 2m, 2m+1
                for m in range(NE):
                    ps = pps.tile([128, SQ], F32, tag="pps", name="ps_q")
                    for half in range(2):
                        sl = slice(512 * half, 512 * half + 512)
                        for c in range(NE):
                            nc.tensor.matmul(
                                ps[:, sl],
                                wq_sb[c][:, 128 * m:128 * m + 128],
                                xq_sb[c][:, sl],
                                start=(c == 0), stop=(c == NE - 1))
                    for hh in range(2):
                        psl = slice(64 * hh, 64 * hh + 64)
                        nc.scalar.activation(
                            out=qst[2 * m + hh][0:64, :],
                            in_=ps[psl, :], func=AF.Identity,
                            bias=bq_sb[psl, m:m + 1])
                # K: two n2 blocks of 1024
                for m in range(NE):
                    for n2 in range(2):
                        ps = pps.tile([128, SQ], F32, tag="pps", name="ps_k")
                        for half in range(2):
                            sl = slice(1024 * n2 + 512 * half,
                                       1024 * n2 + 512 * half + 512)
                            for c in range(NE):
                                nc.tensor.matmul(
                                    ps[:, 512 * half:512 * half + 512],
                                    wk_sb[c][:, 128 * m:128 * m + 128],
                                    xk_sb[c][:, sl],
                                    start=(c == 0), stop=(c == NE - 1))
                        for hh in range(2):
                            psl = slice(64 * hh, 64 * hh + 64)
                            nc.scalar.activation(
                                out=kst[2 * m + hh][0:64,
                                                    1024 * n2:1024 * n2 + 1024],
                                in_=ps[psl, :], func=AF.Identity,
                                bias=bk_sb[psl, m:m + 1])

                # V proj interleaved with stats(0): per round, one V k-pair
                # (8 matmuls, PE) + one stats j-step (4 matmuls + 4 DVE TTR).
                def vproj_ops():
                    for c2 in range(NKT // 2):
                        ps = pps.tile([128, SQ], F32, tag="pps",
                                      name=f"ps_v{c2}")
                        for half in range(2):
                            kc = 2 * c2 + half
                            for c in range(NE):
                                nc.tensor.matmul(
                                    ps[:, 512 * half:512 * half + 512],
                                    xv_sb[c][:, 128 * kc:128 * kc + 128],
                                    wv_sb[c][:, :],
                                    start=(c == 0), stop=(c == NE - 1))
                        for half in range(2):
                            kc = 2 * c2 + half
                            pv = ps[:, 512 * half:512 * half + 512].rearrange(
                                "p (h d) -> p h d", d=64)
                            vv = v_sb[kc].rearrange(
                                "p (h d) -> p h d", d=65)[:, :, 0:64]
                            bvv = bv_sb.rearrange("p (h d) -> p h d", d=64)
                            nc.vector.tensor_tensor(
                                out=vv, in0=pv, in1=bvv, op=ALU.add)
                        yield

                vg = vproj_ops()
                sg = stats_ops(0)
                for _ in range(NKT // 2):
                    next(vg)
                    next(sg)

            # ---- attention phase ----
            with (
                tc.tile_pool(name="mtp", bufs=1) as mtp,
                tc.tile_pool(name="ptp", bufs=3) as ptp,
                tc.tile_pool(name="ptm", bufs=3) as ptmp,
                tc.tile_pool(name="tail", bufs=2) as tailp,
                tc.tile_pool(name="st_ps", bufs=2, space="PSUM") as st_ps,
                tc.tile_pool(name="ot_ps", bufs=1, space="PSUM") as ot_ps,
                tc.tile_pool(name="tailps", bufs=1, space="PSUM") as tailps,
            ):
                mt_sb = [mtp.tile([128, SQ], I16, tag=f"mt{c}",
                                  name=f"mt{c}")
                         for c in range(NKT)]
                # only the first half upfront: 16 x 0.25 MB at once
                # saturates the queues and delays the head-boundary row-64
                # DMAs by up to 7 us; the rest trickle in during head 0
                for c in range(NKT // 2):
                    nc.sync.dma_start(out=mt_sb[c][:, :],
                                      in_=mt[128 * c:128 * c + 128, :])

                def load_mt(c):
                    nc.sync.dma_start(out=mt_sb[c][:, :],
                                      in_=mt[128 * c:128 * c + 128, :])

                def attn_head(h):
                    """S^T/exp/AND per chunk with PV delayed one chunk;
                    decoupled tail. Yield layout: per chunk c: (S-stage,
                    P-stage of c-1); then P(15); ACT drain; normalize;
                    4x out-proj pairs; final bias+DMA."""
                    ot = ot_ps.tile([65, SQ], F32, tag="ot", name=f"ot{h}")
                    pm_prev = None

                    def do_P(pm, c):
                        for qb in range(2):
                            nc.tensor.matmul(
                                ot[:, 512 * qb:512 * qb + 512],
                                v_sb[c][:, 65 * h:65 * h + 65],
                                pm[:, 512 * qb:512 * qb + 512],
                                start=(c == 0), stop=(c == NKT - 1),
                                skip_group_check=True)

                    for c in range(NKT):
                        pt = ptp.tile([128, SQ], BF16, tag="pt", name="pt")
                        pm = ptmp.tile([128, SQ], BF16, tag="pm", name="pm")
                        for qb in range(2):
                            st = st_ps.tile([128, 512], F32, tag="st",
                                            name="st")
                            nc.tensor.matmul(
                                st[:, :],
                                kst[h][:, 128 * c:128 * c + 128],
                                qst[h][:, 512 * qb:512 * qb + 512],
                                start=True, stop=True)
                            nc.scalar.activation(
                                out=pt[:, 512 * qb:512 * qb + 512],
                                in_=st[:, :], func=AF.Exp)
                        nc.vector.tensor_tensor(
                            out=pm.bitcast(I16)[:, :],
                            in0=pt.bitcast(I16)[:, :],
                            in1=mt_sb[c][:, :], op=ALU.bitwise_and)
                        yield  # filler slot (stats j / out-proj of h-1)
                        if pm_prev is not None:
                            do_P(pm_prev, c - 1)
                        pm_prev = pm
                        yield
                    do_P(pm_prev, NKT - 1)
                    # tail 1: drain PV psum (frees ot for head h+1). The
                    # partition remap for K=128 out-proj happens HERE on the
                    # ACT engine (cross-base writes are HW-proven on ACT,
                    # not on DVE): otdr2[64a+d, g] = ot[d, 2g+a].
                    zrow = tailp.tile([1, SQ], F32, tag="zrow",
                                      name=f"zrow{h}")
                    nc.scalar.activation(out=zrow[:, :], in_=ot[64:65, :],
                                         func=AF.Identity)
                    otdr2 = tailp.tile([128, SQ // 2], F32, tag="otdr",
                                       name=f"otdr{h}")
                    for a in range(2):
                        nc.scalar.activation(
                            out=otdr2[64 * a:64 * a + 64, :],
                            in_=ot[0:64, :].rearrange(
                                "p (g a) -> p a g", a=2)[:, a, :],
                            func=AF.Identity)
                    yield
                    # tail 2: normalize (off critical path)
                    rz = tailp.tile([1, SQ], F32, tag="rz", name=f"rz{h}")
                    nc.vector.reciprocal_approx_fast(rz[:, :], zrow[:, :])
                    rzb = tailp.tile([128, SQ], F32, tag="rzb",
                                     name=f"rzb{h}")
                    nc.gpsimd.partition_broadcast(rzb[:, :], rz[:, :],
                                                  channels=128)
                    # otd2 packs scramble-pairs (2m, 2m+1) on the partition
                    # axis so out-proj runs K=128: otd2[64a+d, 128m+t] =
                    # ot[d, 8t+2m+a] / Z[8t+2m+a]
                    otd2 = tailp.tile([128, 512], F16, tag="otd",
                                      name=f"otd{h}")
                    for a in range(2):
                        nc.vector.tensor_tensor(
                            out=otd2[64 * a:64 * a + 64, :].rearrange(
                                "p (m t) -> p m t", m=4)[:, :, :],
                            in0=otdr2[64 * a:64 * a + 64, :].rearrange(
                                "p (t m) -> p m t", m=4)[:, :, :],
                            in1=rzb[64 * a:64 * a + 64, :].rearrange(
                                "p (t m a) -> p a m t", m=4, a=2)[:, a, :, :],
                            op=ALU.mult)
                    yield
                    # tail 3: out projection, spread over filler slots
                    po = tailps.tile([128, 512], F32, tag="po", name=f"po{h}")
                    for m in range(4):
                        nc.tensor.matmul(
                            po[:, :],
                            otd2[:, 128 * m:128 * m + 128],
                            wo16[m][:, :],
                            start=(m == 0), stop=(m == 3),
                            skip_group_check=True)
                        if m % 2 == 1:
                            yield
                    o_sb = tailp.tile([128, E], F32, tag="osb",
                                      name=f"osb{h}")
                    nc.vector.tensor_tensor(
                        out=o_sb[:, :], in0=po[:, :], in1=bo_sb[:, :],
                        op=ALU.add)
                    nc.sync.dma_start(out=out[128 * h:128 * h + 128, :],
                                      in_=o_sb[:, :])
                    yield

                # drive: per head: 16 chunks with stats(h+1) j-steps on
                # chunks 0-7, its tail (transpose + row-64 DMAs) on chunk
                # 8, and head h-1's out-proj on the remaining slots.
                tail_gen = None
                for h in range(H):
                    if h == 0:
                        next(sg)  # stats(0) tail at the proj boundary
                    nsg = stats_ops(h + 1) if h + 1 < H else None
                    ag = attn_head(h)
                    for c in range(NKT):
                        next(ag)      # S(c)
                        if h == 0 and c < NKT // 2:
                            load_mt(NKT // 2 + c)
                        if nsg is not None and c < NQT:
                            next(nsg)     # stats(h+1) j=c
                        elif nsg is not None and c == NQT:
                            next(nsg)     # stats(h+1) tail
                        elif tail_gen is not None:
                            next(tail_gen, None)
                        next(ag)      # P(c-1)
                    if tail_gen is not None:
                        for _ in tail_gen:
                            pass
                    next(ag)          # P(15) + ACT drain of ot
                    next(ag)          # normalize (DVE/GpSimd)
                    tail_gen = ag     # rest: out-proj during next head
                    sg = nsg
                for _ in tail_gen:
                    pass

    nc.compile()
    return nc


_NC = None
_last_in_maps = None


def _get_nc():
    global _NC
    if _NC is None:
        _NC = build_nc()
    return _NC


def kernel(query, key_in, value, mask, Wq, bq, Wk, bk, Wv, bv, Wo, bo):
    query = np.asarray(query, np.float32)
    key_in = np.asarray(key_in, np.float32)
    value = np.asarray(value, np.float32)
    mask = np.asarray(mask)
    Wq = np.asarray(Wq, np.float32)
    Wk = np.asarray(Wk, np.float32)
    Wv = np.asarray(Wv, np.float32)
    Wo = np.asarray(Wo, np.float32)

    sdk = np.sqrt(np.float32(DK))
    wqT = np.ascontiguousarray((Wq * sdk).T)
    wkT = np.ascontiguousarray(Wk.T)
    wvT = np.ascontiguousarray(Wv.T).astype(BF)
    woT = np.ascontiguousarray(Wo.T).astype(BF)
    bqt = np.ascontiguousarray(
        (np.asarray(bq, np.float32) * sdk).reshape(NE, 128).T)
    bkt = np.ascontiguousarray(np.asarray(bk, np.float32).reshape(NE, 128).T)
    bvr = np.ascontiguousarray(
        np.tile(np.asarray(bv, np.float32).reshape(1, E), (128, 1)))
    bor = np.ascontiguousarray(
        np.tile(np.asarray(bo, np.float32).reshape(1, E), (128, 1)))

    in_maps = []
    for c in range(N_CORES):
        b, r = c // 2, c % 2
        q0 = SQ * r
        in_maps.append({
            "xqT": np.ascontiguousarray(query[b, q0:q0 + SQ, :].T),
            "xkT": np.ascontiguousarray(key_in[b].T),
            "xvT": np.ascontiguousarray(value[b].T).astype(BF),
            "mt": np.ascontiguousarray(
                (mask[b, q0:q0 + SQ, :].T != 0)).astype(np.int16)
            * np.int16(-1),
            "mb": ((mask[b, q0:q0 + SQ, :] == 0).astype(np.float32)
                   * np.float32(-57344.0)).astype(ml_dtypes.float8_e5m2),
            "wqT": wqT, "wkT": wkT, "wvT": wvT, "woT": woT,
            "bqt": bqt, "bkt": bkt, "bvr": bvr, "bor": bor,
        })

    nc = _get_nc()
    global _last_in_maps
    _last_in_maps = in_maps
    res = run_bass_kernel_spmd(nc, in_maps, list(range(N_CORES)))

    full = np.empty((B, S, E), np.float32)
    for c in range(N_CORES):
        b, r = c // 2, c % 2
        oc = res.results[c]["out"]
        for h in range(H):
            full[b, 256 * h + 128 * r:256 * h + 128 * r + 128, :] = \
                oc[128 * h:128 * h + 128, :]
    return full
